# revision 1
# baseline (speedup 1.0000x reference)
"""Distributed MLA-style attention on 8 Trainium2 NeuronCores.

Sharding: tensor-parallel over num_heads=24 -> 3 heads per core
(per the sharding hint). Per-core work: shared low-rank projections
(replicated, small), per-head-group attention (scores/softmax/AV), and
the head-sharded slice of o_proj. Partial o_proj outputs are summed
across cores with an on-device all-reduce (psum); the full [B,S,D]
output is returned.
"""
import math

import numpy as np
import jax
import jax.numpy as jnp
from jax.sharding import Mesh, PartitionSpec as P
from jax.experimental.shard_map import shard_map

B, S, D = 4, 2048, 768
H = 24
NOPE, ROPE_D, VD = 32, 16, 32
QHD = NOPE + ROPE_D
QR, KVR = 384, 128
HEAD_DIM = D // H  # 32; softmax scale
NCORES = 8
HLOC = H // NCORES  # 3 heads per core

_SCALE = 1.0 / math.sqrt(HEAD_DIM)


def _rms_norm(x, w, eps=1e-5):
    x32 = x.astype(jnp.float32)
    y = x32 * jax.lax.rsqrt(jnp.mean(x32 * x32, axis=-1, keepdims=True) + eps)
    return y.astype(x.dtype) * w


def _rotate(t, cos, sin):
    # t: [B, S, N, r]; cos/sin: [S, r//2]
    shp = t.shape
    tr = t.astype(jnp.float32).reshape(shp[:-1] + (-1, 2))
    xr, xi = tr[..., 0], tr[..., 1]
    c = cos.reshape(1, shp[1], 1, -1)
    s = sin.reshape(1, shp[1], 1, -1)
    out = jnp.stack([xr * c - xi * s, xr * s + xi * c], axis=-1).reshape(shp)
    return out.astype(t.dtype)


def _body(x, mask, freqs_cos, freqs_sin, Wqa, qa_ln, Wqb_s, Wkva, kv_ln,
          Wkvb_s, Wo_s):
    # Per-shard shapes: Wqb_s [QR, HLOC*QHD], Wkvb_s [KVR, HLOC*(NOPE+VD)],
    # Wo_s [HLOC*VD, D]. Everything else replicated.
    b, s = B, S
    q = _rms_norm(x @ Wqa, qa_ln) @ Wqb_s
    q = q.reshape(b, s, HLOC, QHD).transpose(0, 2, 1, 3)      # [B,hl,S,48]
    q_nope, q_pe = q[..., :NOPE], q[..., NOPE:]
    ckv = x @ Wkva
    c_kv, k_pe = ckv[..., :KVR], ckv[..., KVR:]
    kv = (_rms_norm(c_kv, kv_ln) @ Wkvb_s).reshape(b, s, HLOC, NOPE + VD)
    kv = kv.transpose(0, 2, 1, 3)
    k_nope, v = kv[..., :NOPE], kv[..., NOPE:]                # [B,hl,S,32]
    # Reproduce the reference's swapped-rope exactly: rotated shared k_pe
    # goes on the QUERY side (broadcast over heads); rotated per-head q_pe
    # goes on the KEY side.
    rot_qpe = _rotate(q_pe.transpose(0, 2, 1, 3), freqs_cos, freqs_sin)
    rot_kpe = _rotate(k_pe.reshape(b, s, 1, ROPE_D), freqs_cos, freqs_sin)
    q_pe_f = rot_kpe.transpose(0, 2, 1, 3)                    # [B,1,S,16]
    k_pe_f = rot_qpe.transpose(0, 2, 1, 3)                    # [B,hl,S,16]
    qs = jnp.concatenate(
        [q_nope, jnp.broadcast_to(q_pe_f, (b, HLOC, s, ROPE_D))], axis=-1)
    ks_ = jnp.concatenate([k_nope, k_pe_f], axis=-1)          # [B,hl,S,48]

    outs = []
    for bi in range(b):  # loop batches to bound peak scores memory per core
        scores = jnp.einsum('hqd,hkd->hqk', qs[bi], ks_[bi]) * _SCALE
        scores = scores + mask[0, 0, :s, :s][None]
        attn = jax.nn.softmax(scores.astype(jnp.float32), axis=-1)
        attn = attn.astype(ks_.dtype)
        o = jnp.einsum('hqk,hkd->hqd', attn, v[bi])           # [hl,S,32]
        outs.append(o.transpose(1, 0, 2).reshape(s, HLOC * VD))
    attn_out = jnp.stack(outs, axis=0)                        # [B,S,hl*32]
    partial = attn_out @ Wo_s                                 # [B,S,D]
    return jax.lax.psum(partial, 'h')


_CACHE = {}


def _get_fn():
    if 'fn' in _CACHE:
        return _CACHE['fn']
    devs = jax.devices()[:NCORES]
    mesh = Mesh(np.asarray(devs), ('h',))
    rep = P()
    in_specs = (rep, rep, rep, rep, rep, rep,
                P(None, 'h'),        # Wqb reshaped [QR, H, QHD] -> flat below
                rep, rep,
                P(None, 'h'),        # Wkvb
                P('h', None))        # Wo
    fn = jax.jit(shard_map(_body, mesh=mesh, in_specs=in_specs,
                           out_specs=rep, check_rep=False))
    _CACHE['fn'] = (fn, mesh)
    return _CACHE['fn']


def kernel(x, mask, freqs_cos, freqs_sin, Wqa, qa_ln, Wqb, Wkva, kv_ln,
           Wkvb, Wo):
    fn, mesh = _get_fn()
    # Reorder weight columns so a contiguous split over axis gives whole
    # heads: Wqb [QR, H*QHD] is already head-major; same for Wkvb and Wo.
    out = fn(jnp.asarray(x), jnp.asarray(mask), jnp.asarray(freqs_cos),
             jnp.asarray(freqs_sin), jnp.asarray(Wqa), jnp.asarray(qa_ln),
             jnp.asarray(Wqb), jnp.asarray(Wkva), jnp.asarray(kv_ln),
             jnp.asarray(Wkvb), jnp.asarray(Wo))
    return np.asarray(jax.block_until_ready(out)).astype(np.float32)


if __name__ == '__main__':
    rng = np.random.default_rng(0)
    ins = dict(
        x=rng.standard_normal((B, S, D), np.float32),
        mask=np.zeros((1, 1, S, S), np.float32),
        freqs_cos=rng.random((S, ROPE_D // 2), np.float32),
        freqs_sin=rng.random((S, ROPE_D // 2), np.float32),
        Wqa=rng.standard_normal((D, QR), np.float32) * D ** -0.5,
        qa_ln=np.ones((QR,), np.float32),
        Wqb=rng.standard_normal((QR, H * QHD), np.float32) * QR ** -0.5,
        Wkva=rng.standard_normal((D, KVR + ROPE_D), np.float32) * D ** -0.5,
        kv_ln=np.ones((KVR,), np.float32),
        Wkvb=rng.standard_normal((KVR, H * (NOPE + VD)), np.float32) * KVR ** -0.5,
        Wo=rng.standard_normal((H * VD, D), np.float32) * (H * VD) ** -0.5,
    )
    out = kernel(**ins)
    print('kernel out', out.shape, out.dtype, float(np.abs(out).max()))
